# revision 28
# baseline (speedup 1.0000x reference)
"""Trainium2 Bass kernel for GNN message passing.

Computes, for full inputs:
    edge_h = lrelu(lrelu(edge @ We1 + be1) @ We2 + be2)        # [E, 128]
    out    = segment_sum((node @ Wn)[idx_j] * edge_h, seg_i)   # [N, 128]

Strategy (8 NeuronCores, SPMD single program):
  - Destination permutation (host): nodes re-labelled so each of the 392
    (core, window) bins holds ~equal edge load -> near-zero schedule
    padding, perfect core balance. Output rows un-permuted on host.
  - Shard edges by permuted DESTINATION range (6250 nodes per core); each
    core produces its own output slice -> no collectives.
  - Phase 1 (replicated): msg = node @ Wn written to two internal DRAM
    tables (asymmetric split A=17408 / B=32768 rows so gather indices fit
    int16 and pass-A gathers can start after only 17 slabs), partition-
    major fp16. Table membership is chosen PER CORE (each core stages its
    own sigma-permuted nodeT) to balance per-window A/B edge counts to
    the 4+8 tile caps.
  - Phase 2, two passes (A then B). Per 128-edge tile: edge MLP via PE
    matmuls, LeakyReLU on the scalar engine, product on DVE (batched once
    per gather run), scatter-add = one-hot matmul into a PSUM window
    [128 slots x 128 feat]. One-hots are precomputed on host and DMA'd in
    on the sync queue (no DVE is_equal). Pass-A windows stash partials in
    SBUF; pass-B adds and stores.
  - Gathers (InstDMAGatherAnt) round-robin on 4 SWDGE queues; the gpsimd
    queue is reserved for gathers so gather issue is never blocked.
    Phase-1 loads + in-loop phase-2 input loads go on the sync queue; the
    first runs' inputs and idx table are prefetched on the scalar queue
    so phase 1 starts immediately. Table-B phase-1 slabs are interleaved
    (front-loaded) with pass-A compute in PE program order.
"""

import os
import sys
import types

import numpy as np

import concourse.bass as bass
import concourse.tile as tile
from concourse.tile import add_dep_helper
from concourse import bacc, mybir
from concourse.bass_utils import run_bass_kernel_spmd

# ---- problem constants (hardcoded per spec) ----
N_NODES = 50000
D_NODE = 128
D_EDGE = 32
D_HID = 128
N_CORES = 8
NPC = N_NODES // N_CORES          # nodes per core = 6250
P = 128                           # partitions
W_PER_CORE = (NPC + P - 1) // P   # 49 windows per core
LAST_W_CAP = NPC - (W_PER_CORE - 1) * P   # 106 node slots in window 48
NEG_SLOPE = 0.01
PAD_SLOT = 300.0                  # slot value that never matches 0..127

N_PAD = 50176                     # 49 x 1024
HALF_A = 17 * 1024                # msg table A rows (17408)
HALF_B = 32 * 1024                # msg table B rows (32768, int16 max fit)
HALVES = (HALF_A, HALF_B)
CAPS = (512, 1024)                # per-window edge caps (4 + 8 tiles)
TPBS = (HALF_A // P, HALF_B // P)  # 136, 256 (partition-major tables)

RUN_MAX = 8          # max tiles per dma_gather (1024 idxs; ring cap)
N_QUEUES = 4         # SWDGE queues for parallel desc-gen
PREFETCH = 6         # phase-2 input runs loaded before phase-1 loads

F32 = mybir.dt.float32
F16 = mybir.dt.float16
I16 = mybir.dt.int16
DT = F16
DT_NP = np.float16

TRACE = False
LAST_RESULT = None

_PROGRAM_CACHE = {}


def _ensure_ntff_hook():
    """Provide antenv.axon_hooks if this image's antenv lacks it, and
    register the ctypes NTFF profiling hook so trace=True works."""
    try:
        from antenv.axon_hooks import get_axon_ntff_profile_hook  # noqa: F401
        return
    except ImportError:
        pass
    mod = types.ModuleType("antenv.axon_hooks")
    _hook = [None]
    mod.set_axon_ntff_profile_hook = lambda h: _hook.__setitem__(0, h)
    mod.get_axon_ntff_profile_hook = lambda: _hook[0]
    sys.modules["antenv.axon_hooks"] = mod
    import antenv

    antenv.axon_hooks = mod
    try:
        from trn_agent_boot.trn_boot import _ntff_profile_via_ctypes

        mod.set_axon_ntff_profile_hook(
            _ntff_profile_via_ctypes("/opt/axon/libaxon_pjrt.so")
        )
    except Exception:
        pass


# --------------------------------------------------------------------------
# host-side balancing permutations
# --------------------------------------------------------------------------

def cdiv(a, b):
    return (a + b - 1) // b


def _balance_dest(seg_i):
    """Permutation pi (orig node id -> new id) equalizing per-(core,window)
    edge loads via LPT greedy. New id = core*NPC + win*P + slot."""
    import heapq

    deg = np.bincount(seg_i, minlength=N_NODES).astype(np.int64)
    order = np.argsort(-deg, kind="stable")
    NB = N_CORES * W_PER_CORE
    caps = np.empty(NB, dtype=np.int64)
    for c in range(N_CORES):
        for w in range(W_PER_CORE):
            caps[c * W_PER_CORE + w] = P if w < W_PER_CORE - 1 else LAST_W_CAP
    fill = np.zeros(NB, dtype=np.int64)
    loads = np.zeros(NB, dtype=np.int64)
    heap = [(0, int(b)) for b in range(NB)]
    heapq.heapify(heap)
    pi = np.empty(N_NODES, dtype=np.int64)
    for n in order:
        while True:
            load, b = heapq.heappop(heap)
            if fill[b] >= caps[b]:
                continue
            if load != loads[b]:
                heapq.heappush(heap, (int(loads[b]), b))
                continue
            break
        c, w = divmod(b, W_PER_CORE)
        pi[n] = c * NPC + w * P + fill[b]
        fill[b] += 1
        loads[b] += deg[n]
        if fill[b] < caps[b]:
            heapq.heappush(heap, (int(loads[b]), b))
    assert (fill == caps).all()
    return pi


def _assign_tables(seg2, idx_j):
    """Per-core A/B table assignment of source nodes (A:B edge ratio 1:2).

    Returns half[e] (0 if edge's source is in table A for its core) and
    row_of[c] : [N_NODES] -> row in 0..N_PAD (A: 0..HALF_A-1, B: HALF_A..),
    node_at_row[c] : [N_PAD] node id at row (-1 = pad).
    """
    core = seg2 // NPC
    win = (seg2 % NPC) // P
    E = len(seg2)
    half = np.zeros(E, dtype=np.int64)
    node_at_row = np.full((N_CORES, N_PAD), -1, dtype=np.int64)
    row_of = np.full((N_CORES, N_NODES), -1, dtype=np.int64)
    CAP_A, CAP_B = CAPS

    for c in range(N_CORES):
        m = np.flatnonzero(core == c)
        srcs = idx_j[m]
        wins = win[m]
        order = np.argsort(srcs, kind="stable")
        s_sorted = srcs[order]
        w_sorted = wins[order]
        uniq, starts = np.unique(s_sorted, return_index=True)
        counts = np.diff(np.append(starts, len(s_sorted)))
        A_cnt = np.zeros(W_PER_CORE, np.int64)
        B_cnt = np.zeros(W_PER_CORE, np.int64)
        sideA = np.zeros(len(uniq), dtype=bool)

        multi = np.flatnonzero(counts > 1)
        multi = multi[np.argsort(-counts[multi], kind="stable")]
        for ui in multi:
            s0 = starts[ui]
            ws = w_sorted[s0:s0 + counts[ui]]
            uw, uc = np.unique(ws, return_counts=True)
            fa = (A_cnt[uw] + uc).max() / CAP_A
            fb = (B_cnt[uw] + uc).max() / CAP_B
            okA = fa <= 1.0
            okB = fb <= 1.0
            useA = fa <= fb if (okA == okB) else okA
            if useA:
                A_cnt[uw] += uc
                sideA[ui] = True
            else:
                B_cnt[uw] += uc

        # single-edge sources: per window, top up A toward tot/3
        single = np.flatnonzero(counts == 1)
        sw = w_sorted[starts[single]]
        for w in range(W_PER_CORE):
            sel = single[sw == w]
            n_s = len(sel)
            tot = A_cnt[w] + B_cnt[w] + n_s
            want_a = int(np.clip(tot // 3 - A_cnt[w], 0, n_s))
            want_a = min(want_a, max(CAP_A - A_cnt[w], 0))
            spill_b = (B_cnt[w] + n_s - want_a) - CAP_B
            if spill_b > 0:
                want_a = min(want_a + spill_b, n_s)
            if want_a:
                sideA[sel[:want_a]] = True
                A_cnt[w] += want_a
            B_cnt[w] += n_s - want_a

        # tables hold ONLY this core's active sources (rows above the
        # active count are never gathered, so their slabs are skipped)
        a_list = uniq[sideA]
        b_list = uniq[~sideA]
        assert len(a_list) <= HALF_A and len(b_list) <= HALF_B, (
            f"core {c}: |A|={len(a_list)} |B|={len(b_list)}"
        )
        node_at_row[c, :len(a_list)] = a_list
        node_at_row[c, HALF_A:HALF_A + len(b_list)] = b_list
        row_of[c, a_list] = np.arange(len(a_list))
        row_of[c, b_list] = HALF_A + np.arange(len(b_list))
        half[m] = (row_of[c, srcs] >= HALF_A).astype(np.int64)

    # slabs actually needed per table (max active count over cores)
    n_used = [
        int((node_at_row[:, :HALF_A] >= 0).sum(axis=1).max()),
        int((node_at_row[:, HALF_A:] >= 0).sum(axis=1).max()),
    ]
    n_slabs = (cdiv(n_used[0], 1024), cdiv(n_used[1], 1024))
    return half, node_at_row, row_of, n_slabs


# --------------------------------------------------------------------------
# host-side schedule + packing
# --------------------------------------------------------------------------

class Schedule:
    """Common (all-core) static schedule baked into the program.

    Tile sequence = pass A (all windows, table 0) then pass B (table 1).
    """

    def __init__(self, seg2, half):
        core = seg2 // NPC
        local = seg2 - core * NPC
        win = local // P

        cnt = np.zeros((2, N_CORES, W_PER_CORE), dtype=np.int64)
        for c in range(N_CORES):
            m = core == c
            for h in (0, 1):
                cnt[h, c] = np.bincount(
                    win[m & (half == h)], minlength=W_PER_CORE
                )
        a_tiles = np.maximum(cdiv(cnt[0].max(axis=0), P), 1)
        b_tiles = np.maximum(cdiv(cnt[1].max(axis=0), P), 1)

        win_of, table_of = [], []
        self.block_start = np.zeros((W_PER_CORE, 2), dtype=np.int64)
        self.block_tiles = np.zeros((W_PER_CORE, 2), dtype=np.int64)
        for tab in (0, 1):
            nts = a_tiles if tab == 0 else b_tiles
            for w in range(W_PER_CORE):
                self.block_start[w, tab] = len(win_of)
                self.block_tiles[w, tab] = nts[w]
                for _ in range(int(nts[w])):
                    win_of.append(w)
                    table_of.append(tab)
        win_of = np.array(win_of, dtype=np.int64)
        table_of = np.array(table_of, dtype=np.int64)
        T = len(win_of)
        first_of = np.zeros(T, dtype=bool)
        last_of = np.zeros(T, dtype=bool)
        for w in range(W_PER_CORE):
            for tab in (0, 1):
                s = int(self.block_start[w, tab])
                n = int(self.block_tiles[w, tab])
                first_of[s] = True
                last_of[s + n - 1] = True

        self.T = T
        self.win_of = win_of
        self.table_of = table_of
        self.first_of = first_of
        self.last_of = last_of
        self.core, self.local, self.win, self.half = core, local, win, half
        self.n_a_tiles = int(a_tiles.sum())

        # gather runs: same-table spans capped at RUN_MAX
        runs = []
        t = 0
        while t < T:
            tab = self.table_of[t]
            e = t
            while e < T and self.table_of[e] == tab and e - t < RUN_MAX:
                e += 1
            runs.append((int(tab), t, e - t))
            t = e
        self.runs = runs

        # chunks: <=4-tile pieces within runs
        chunks = []
        for ri, (tab, t0, L) in enumerate(runs):
            t = t0
            while t < t0 + L:
                nt = min(4, t0 + L - t)
                chunks.append((ri, t, nt))
                t += nt
        self.chunks = chunks

    def key(self):
        return (
            tuple(self.win_of.tolist()),
            tuple(self.table_of.tolist()),
            self.n_slabs,
        )


def _pack_core(c, S, edge, idx_j, row_of_c):
    """Per-core padded arrays following the common schedule."""
    T = S.T
    perm = np.full(T * P, -1, dtype=np.int64)
    for w in range(W_PER_CORE):
        for tab in (0, 1):
            sel = np.flatnonzero(
                (S.core == c) & (S.win == w) & (S.half == tab)
            )
            n = len(sel)
            s0 = S.block_start[w, tab] * P
            cap = S.block_tiles[w, tab] * P
            assert n <= cap, f"schedule overflow c={c} w={w} tab={tab}"
            perm[s0:s0 + n] = sel

    valid = perm >= 0
    pidx = np.where(valid, perm, 0)
    tab_of_pos = S.table_of[np.arange(T * P) // P]

    # idx16 [128, T*8]: per tile-order flattening i=t*128+p -> [i%16, i//16]
    loc = (row_of_c[idx_j[pidx]] - tab_of_pos * HALF_A).astype(np.int64)
    loc[~valid] = 0
    tpb = np.where(tab_of_pos == 0, TPBS[0], TPBS[1])
    assert (loc >= 0).all() and (loc < np.where(tab_of_pos == 0, HALF_A, HALF_B)).all()
    # msg tables partition-major: row j at position (j%128)*TPB + j//128
    loc = (loc % P) * tpb + loc // P
    assert loc.max() <= 32767
    idx16 = loc.astype(np.int16).reshape(T * 8, 16).T  # [16, T*8]
    idx16 = np.tile(idx16, (8, 1)).copy()              # [128, T*8]

    # chunk-packed edge features, slots -> host-built one-hots
    n_ch = len(S.chunks)
    edgeT = np.zeros((n_ch, D_EDGE, 512), dtype=DT_NP)
    slots_all = S.local[pidx] % P
    ev = np.zeros((T * P, D_EDGE), dtype=DT_NP)
    ev[valid] = edge[pidx[valid]]
    for k, (ri, t0, nt) in enumerate(S.chunks):
        edgeT[k, :, :nt * P] = ev[t0 * P:(t0 + nt) * P].T

    # one-hot, edge-position-major: ohT[p, t*128 + s] = (slot(t,p) == s)
    oh = (slots_all[:, None] == np.arange(P)[None, :])
    oh &= valid[:, None]
    ohT = np.ascontiguousarray(
        oh.reshape(T, P, P).transpose(1, 0, 2).reshape(P, T * P)
    ).astype(DT_NP)
    return edgeT, ohT, idx16


# --------------------------------------------------------------------------
# device program
# --------------------------------------------------------------------------

def _build_program(S):
    T = S.T
    n_ch = len(S.chunks)
    dbg_skip_p1 = os.environ.get("K_SKIP_P1") == "1"
    dbg_skip_gather = os.environ.get("K_SKIP_GATHER") == "1"
    dbg_no_gate = os.environ.get("K_NO_GATE") == "1"

    nc = bacc.Bacc(
        "TRN2", target_bir_lowering=False, debug=False, num_devices=N_CORES,
        num_swdge_queues=N_QUEUES,
    )

    # ---- I/O ----
    nodeT_h = nc.dram_tensor("nodeT", [P, N_PAD], DT, kind="ExternalInput").ap()
    Wn_h = nc.dram_tensor("Wn", [D_NODE, D_HID], DT, kind="ExternalInput").ap()
    We1_h = nc.dram_tensor("We1p", [D_EDGE, D_HID], DT, kind="ExternalInput").ap()
    We2_h = nc.dram_tensor("We2", [D_HID, D_HID], DT, kind="ExternalInput").ap()
    be1_h = nc.dram_tensor("be1c", [P, 1], F32, kind="ExternalInput").ap()
    be2_h = nc.dram_tensor("be2bc", [P, 512], F32, kind="ExternalInput").ap()
    edgeT_h = nc.dram_tensor(
        "edgeT", [n_ch, D_EDGE, 512], DT, kind="ExternalInput"
    ).ap()
    ohT_h = nc.dram_tensor(
        "ohT", [P, T * P], DT, kind="ExternalInput"
    ).ap()
    idx16_h = nc.dram_tensor(
        "idx16", [P, T * 8], I16, kind="ExternalInput"
    ).ap()
    out_h = nc.dram_tensor(
        "out", [W_PER_CORE * P, D_HID], F32, kind="ExternalOutput"
    ).ap()

    msg_h = [
        nc.dram_tensor("msgA", [HALF_A, D_HID], DT).ap(),
        nc.dram_tensor("msgB", [HALF_B, D_HID], DT).ap(),
    ]

    LR = mybir.ActivationFunctionType.Prelu

    with tile.TileContext(nc) as tc:
        with tc.tile_pool(name="consts", bufs=1) as cpool:
            Wn_sb = cpool.tile([D_NODE, D_HID], DT)
            nc.sync.dma_start(Wn_sb[:], Wn_h[:])
            We1_sb = cpool.tile([D_EDGE, D_HID], DT)
            We2_sb = cpool.tile([D_HID, D_HID], DT)
            be1_sb = cpool.tile([P, 1], F32)
            be2_sb = cpool.tile([P, 512], F32)
            ix_all = cpool.tile([P, T * 8], I16)
            nc.scalar.dma_start(ix_all[:], idx16_h[:])

            with (
                tc.tile_pool(name="p2_in", bufs=8) as p2in,
                tc.tile_pool(name="p2_oh", bufs=PREFETCH + 4) as ohpool,
            ):
                # prefetch the first PREFETCH runs' phase-2 inputs ahead of
                # the phase-1 load train on the sync queue
                chunks_by_run = {}
                for k, (ri, t0, nt) in enumerate(S.chunks):
                    chunks_by_run.setdefault(ri, []).append((k, t0, nt))

                et_tiles, oh_tiles = {}, {}

                def load_run_inputs(ri, eng):
                    tab, rt0, L = S.runs[ri]
                    rchunks = chunks_by_run[ri]
                    k0 = rchunks[0][0]
                    nk = len(rchunks)
                    et = p2in.tile([D_EDGE, 2 * 512], DT, tag="edgeT")
                    eng.dma_start(
                        et[:, :nk * 512].rearrange("e (k c) -> e k c", k=nk),
                        edgeT_h[k0:k0 + nk, :, :].rearrange(
                            "k e c -> e k c"
                        ),
                    )
                    oh = ohpool.tile([P, RUN_MAX * P], DT, tag="oh")
                    eng.dma_start(
                        oh[:, :L * P], ohT_h[:, rt0 * P:(rt0 + L) * P]
                    )
                    et_tiles[ri] = et
                    oh_tiles[ri] = oh

                # prefetch on the scalar queue: the sync queue must start
                # the phase-1 load train immediately
                for ri in range(min(PREFETCH, len(S.runs))):
                    load_run_inputs(ri, nc.scalar)

                slabs = []
                for tab in (0, 1):
                    for k in range(S.n_slabs[tab]):
                        slabs.append((tab, k * 1024))
                if dbg_skip_p1:
                    slabs = []
                p1_stores = [[], []]
                msg_sems = [
                    nc.alloc_semaphore("msgA_done"),
                    nc.alloc_semaphore("msgB_done"),
                ]

                # ---- phase 1 + 2 (B slabs interleaved with pass A) ----
                with (
                    tc.tile_pool(name="p1_in", bufs=12) as p1in,
                    tc.tile_pool(name="p1_stage", bufs=8) as p1st,
                    tc.tile_pool(name="p2_g", bufs=16) as p2g,
                    tc.tile_pool(name="p2_mid", bufs=6) as p2mid,
                    tc.tile_pool(name="p2_acc", bufs=1) as accp,
                    tc.tile_pool(name="big_psum", bufs=2, space="PSUM") as bigps,
                    tc.tile_pool(name="h2_psum", bufs=2, space="PSUM") as h2ps,
                    tc.tile_pool(name="out_psum", bufs=2, space="PSUM") as outps,
                    tc.tile_pool(name="out_stage", bufs=3) as outst,
                ):
                    def emit_slab(g):
                        tab, r0 = slabs[g]
                        col0 = tab * HALF_A + r0
                        nt_sb = p1in.tile([P, 1024], DT, tag="nodeT")
                        nc.sync.dma_start(
                            nt_sb[:], nodeT_h[:, col0:col0 + 1024]
                        )
                        ps = bigps.tile([P, 1024], F32, tag="big")
                        for t in range(1024 // P):
                            nc.tensor.matmul(
                                ps[:, t * P:(t + 1) * P],
                                lhsT=nt_sb[:, t * P:(t + 1) * P],
                                rhs=Wn_sb[:],
                                start=True,
                                stop=True,
                            )
                        stage = p1st.tile([P, 1024], DT, tag="p1stage")
                        if g % 2 == 0:
                            nc.vector.tensor_copy(stage[:], ps[:])
                        else:
                            nc.scalar.activation(
                                stage[:], ps[:],
                                mybir.ActivationFunctionType.Copy,
                            )
                        # partition-major: row j -> (j%128)*TPB + j//128
                        t0 = r0 // P
                        dst = msg_h[tab][:].rearrange(
                            "(p t) f -> p t f", t=TPBS[tab]
                        )[:, t0:t0 + 8, :]
                        srcap = stage[:].rearrange("p (t f) -> p t f", t=8)
                        st_inst = nc.scalar.dma_start(dst, srcap)
                        p1_stores[tab].append(st_inst.ins)

                    sem_target = [0, 0]

                    def store_barrier(tab):
                        # Cycling the p1stage ring with dummy writes forces
                        # pool-WAR waits on the last 8 stores' completions;
                        # ring recycling orders all earlier stores before
                        # those transitively. The then_inc on the dummy
                        # compute ops is reliable (unlike DMA then_inc or
                        # shared-lane dep counting).
                        nc.scalar.drain(fusable=False).then_inc(
                            msg_sems[tab], 1
                        )
                        n = min(8, len(p1_stores[tab]))
                        for _ in range(n):
                            dmy = p1st.tile([P, 1024], DT, tag="p1stage")
                            nc.vector.tensor_copy(dmy[:1, :1], Wn_sb[:1, :1])
                        # in-order DVE: this inc fires after all dummies
                        nc.vector.sem_inc(msg_sems[tab], 1)
                        sem_target[tab] = 2

                    n_a_slabs = S.n_slabs[0]
                    for g in range(min(n_a_slabs, len(slabs))):
                        emit_slab(g)
                    emitted = [min(n_a_slabs, len(slabs))]
                    if p1_stores[0]:
                        store_barrier(0)
                    # phase-2 consts: issued after the A-slab load train,
                    # well before first use (~65us)
                    nc.sync.dma_start(We1_sb[:], We1_h[:])
                    nc.sync.dma_start(We2_sb[:], We2_h[:])
                    nc.sync.dma_start(be1_sb[:], be1_h[:])
                    nc.sync.dma_start(be2_sb[:], be2_h[:])

                    def emit_b_slabs(upto):
                        while emitted[0] < min(upto, len(slabs)):
                            emit_slab(emitted[0])
                            emitted[0] += 1
                            if emitted[0] == len(slabs):
                                store_barrier(1)
                    cur_out = {}
                    acc = {}
                    reg_full = nc.gpsimd.to_reg(RUN_MAX * P)

                    first_run_of_tab = {}
                    for ri, (tab, rt0, L) in enumerate(S.runs):
                        if tab not in first_run_of_tab:
                            first_run_of_tab[tab] = ri
                    n_a_runs = max(first_run_of_tab.get(1, len(S.runs)), 1)
                    n_b_slabs = len(slabs) - emitted[0]

                    def emit_scatter(job):
                        ri, tab, rt0, L, oh_run, pr_run = job
                        # out_w[s,f] += onehot[:,t].T @ product[:,t]
                        for t in range(L):
                            i = rt0 + t
                            w = int(S.win_of[i])
                            if S.first_of[i]:
                                cur_out[w] = outps.tile(
                                    [P, P], F32, tag="outp",
                                    name=f"outp_w{w}t{tab}"
                                )
                            nc.tensor.matmul(
                                cur_out[w][:],
                                lhsT=oh_run[:, t * P:(t + 1) * P],
                                rhs=pr_run[:, t * P:(t + 1) * P],
                                start=bool(S.first_of[i]),
                                stop=bool(S.last_of[i]),
                            )
                            if S.last_of[i]:
                                if tab == 0:
                                    a = accp.tile(
                                        [P, P], F32, tag=f"acc_w{w}",
                                        name=f"acc_w{w}"
                                    )
                                    nc.vector.tensor_copy(a[:], cur_out[w][:])
                                    acc[w] = a
                                else:
                                    st = outst.tile(
                                        [P, P], F32, tag="outstage",
                                        name=f"outst_w{w}"
                                    )
                                    nc.vector.tensor_tensor(
                                        st[:], in0=cur_out[w][:],
                                        in1=acc[w][:],
                                        op=mybir.AluOpType.add,
                                    )
                                    nc.sync.dma_start(
                                        out_h[w * P:(w + 1) * P, :], st[:]
                                    )
                                del cur_out[w]

                    pending = []
                    for ri, (tab, rt0, L) in enumerate(S.runs):
                        if tab == 0:
                            # interleave table-B phase-1 slabs with pass A:
                            # ~10 before run 0 (PE is head-of-line blocked
                            # on gather-0 data there), the rest spread 2x
                            # front-loaded so msgB is ready when pass-A
                            # gathers drain
                            base = min(20, n_b_slabs)
                            quota = n_a_slabs + base + (
                                2 * (ri + 1) * (n_b_slabs - base)
                            ) // max(n_a_runs, 1)
                            emit_b_slabs(quota)
                        else:
                            emit_b_slabs(len(slabs))

                        if ri == first_run_of_tab.get(tab):
                            # gpsimd-queue barrier on the store-completion
                            # proof (drain + stage-ring WAR dummies)
                            if p1_stores[tab]:
                                nc.gpsimd.wait_ge(
                                    msg_sems[tab], sem_target[tab]
                                )
                        G = p2g.tile(
                            [P, RUN_MAX * P], DT, tag="G", name=f"G_r{ri}"
                        )
                        if dbg_skip_gather:
                            nc.gpsimd.memset(G[:, :L * P], 0.5)
                        else:
                            g_inst = nc.gpsimd.dma_gather(
                                G[:, :L * P].rearrange(
                                    "p (g f) -> p g f", f=P
                                ),
                                msg_h[tab][:],
                                ix_all[:, rt0 * 8:(rt0 + L) * 8],
                                num_idxs=L * P,
                                num_idxs_reg=(
                                    reg_full if L == RUN_MAX else L * P
                                ),
                                elem_size=P,
                                elem_step=P,
                                queue_num=ri % N_QUEUES,
                            )
                            # ordering vs msg stores is enforced by the
                            # wait_ge barrier above; per-store dep edges
                            # lower to shared-lane counts that fire early
                            # (races) or late (stalls) and must not be used


                        if ri not in et_tiles:
                            load_run_inputs(ri, nc.sync)
                        et_run = et_tiles.pop(ri)
                        oh_run = oh_tiles.pop(ri)
                        if ri + PREFETCH < len(S.runs):
                            load_run_inputs(ri + PREFETCH, nc.sync)

                        eh_run = p2mid.tile([P, RUN_MAX * P], DT, tag="eh")
                        pr_run = p2mid.tile([P, RUN_MAX * P], DT, tag="pr")
                        y_run = p2mid.tile([P, RUN_MAX * P], DT, tag="y")

                        # h1 = lrelu(edge @ We1 + be1), [h x e], whole run
                        # (matmul output must stay within one PSUM bank ->
                        #  one 512-col matmul per chunk)
                        ps1 = bigps.tile([P, 1024], F32, tag="big")
                        for (k, t0, nt) in chunks_by_run[ri]:
                            kk = k - chunks_by_run[ri][0][0]
                            nc.tensor.matmul(
                                ps1[:, kk * 512:kk * 512 + nt * P],
                                lhsT=We1_sb[:],
                                rhs=et_run[:, kk * 512:kk * 512 + nt * P],
                                start=True,
                                stop=True,
                            )
                        h1f = p2mid.tile([P, RUN_MAX * P], DT, tag="h1f")
                        nc.scalar.activation(
                            h1f[:, :L * P], ps1[:, :L * P], LR,
                            bias=be1_sb[:], scale=1.0, alpha=NEG_SLOPE,
                        )

                        for (k, t0, nt) in chunks_by_run[ri]:
                            ncols = nt * P
                            kk = k - chunks_by_run[ri][0][0]

                            # h2 = h1.T @ We2 + be2, edge-major [e x h]
                            ps2 = h2ps.tile([P, 512], F32, tag="h2ps")
                            for t in range(nt):
                                tt = kk * 4 + t
                                nc.tensor.matmul(
                                    ps2[:, t * P:(t + 1) * P],
                                    lhsT=h1f[:, tt * P:(tt + 1) * P],
                                    rhs=We2_sb[:],
                                    start=True,
                                    stop=True,
                                )
                            # bias add downcasts to fp16 SBUF (cheap act in)
                            nc.vector.tensor_tensor(
                                y_run[:, kk * 512:kk * 512 + ncols],
                                in0=ps2[:, :ncols],
                                in1=be2_sb[:, :ncols], op=mybir.AluOpType.add,
                            )

                        # eh = lrelu(y), whole run in one scalar op
                        nc.scalar.activation(
                            eh_run[:, :L * P], y_run[:, :L * P], LR,
                            scale=1.0, alpha=NEG_SLOPE,
                        )

                        # product = gathered msg * edge_h, whole run at once
                        nc.vector.tensor_tensor(
                            pr_run[:, :L * P],
                            in0=G[:, :L * P],
                            in1=eh_run[:, :L * P],
                            op=mybir.AluOpType.mult,
                        )

                        # scatter is emitted one run late (software
                        # pipeline): the in-order PE queue would otherwise
                        # head-of-line block the next run's edge MLP on
                        # this run's gather data
                        pending.append((ri, tab, rt0, L, oh_run, pr_run))
                        # stagger depth 3 during the gather ramp, ramped
                        # down near the end: late scatters have their data
                        # ready, and holding them only lengthens the tail
                        depth = 3 if ri < len(S.runs) - 4 else 1
                        while len(pending) > depth:
                            emit_scatter(pending.pop(0))

                    while pending:
                        emit_scatter(pending.pop(0))

    nc.compile()
    return nc


# --------------------------------------------------------------------------
# entry point
# --------------------------------------------------------------------------

def kernel(node, edge, Wn, We1, be1, We2, be2, seg_i, idx_j):
    global LAST_RESULT
    node = np.asarray(node, dtype=np.float32)
    edge = np.asarray(edge, dtype=np.float32)
    Wn = np.asarray(Wn, dtype=np.float32)
    We1 = np.asarray(We1, dtype=np.float32)
    be1 = np.asarray(be1, dtype=np.float32)
    We2 = np.asarray(We2, dtype=np.float32)
    be2 = np.asarray(be2, dtype=np.float32)
    seg_i = np.asarray(seg_i, dtype=np.int32).astype(np.int64)
    idx_j = np.asarray(idx_j, dtype=np.int32).astype(np.int64)

    pi = _balance_dest(seg_i)
    seg2 = pi[seg_i]
    half, node_at_row, row_of, n_slabs = _assign_tables(seg2, idx_j)
    S = Schedule(seg2, half)
    S.n_slabs = n_slabs
    key = S.key()
    if key not in _PROGRAM_CACHE:
        _PROGRAM_CACHE[key] = _build_program(S)
    nc = _PROGRAM_CACHE[key]

    common = {
        "Wn": Wn.astype(DT_NP),
        "We1p": We1.astype(DT_NP),
        "We2": We2.astype(DT_NP),
        "be1c": be1.reshape(P, 1).copy(),
        "be2bc": np.broadcast_to(
            np.tile(be2, 4), (P, 512)
        ).astype(np.float32).copy(),
    }
    nodeT_f16 = node.T.astype(DT_NP)   # [128, N_NODES]
    in_maps = []
    for c in range(N_CORES):
        edgeT, ohT, idx16 = _pack_core(c, S, edge, idx_j, row_of[c])
        nodeT = np.zeros((P, N_PAD), dtype=DT_NP)
        rows = node_at_row[c]
        m = rows >= 0
        nodeT[:, m] = nodeT_f16[:, rows[m]]
        mp = dict(common)
        mp["nodeT"] = nodeT
        mp["edgeT"] = edgeT
        mp["ohT"] = ohT
        mp["idx16"] = idx16
        in_maps.append(mp)

    if TRACE:
        _ensure_ntff_hook()
    res = run_bass_kernel_spmd(
        nc, in_maps, list(range(N_CORES)), trace=TRACE
    )
    LAST_RESULT = res
    out_cat = np.concatenate(
        [res.results[c]["out"][:NPC] for c in range(N_CORES)], axis=0
    )
    return out_cat[pi].astype(np.float32)


# revision 29
# speedup vs baseline: 1.0240x; 1.0240x over previous
"""Trainium2 Bass kernel for GNN message passing.

Computes, for full inputs:
    edge_h = lrelu(lrelu(edge @ We1 + be1) @ We2 + be2)        # [E, 128]
    out    = segment_sum((node @ Wn)[idx_j] * edge_h, seg_i)   # [N, 128]

Strategy (8 NeuronCores, SPMD single program):
  - Destination permutation (host): nodes re-labelled so each of the 392
    (core, window) bins holds ~equal edge load -> near-zero schedule
    padding, perfect core balance. Output rows un-permuted on host.
  - Shard edges by permuted DESTINATION range (6250 nodes per core); each
    core produces its own output slice -> no collectives.
  - Phase 1 (replicated): msg = node @ Wn written to two internal DRAM
    tables (asymmetric split A=17408 / B=32768 rows so gather indices fit
    int16 and pass-A gathers can start after only 17 slabs), partition-
    major fp16. Table membership is chosen PER CORE (each core stages its
    own sigma-permuted nodeT) to balance per-window A/B edge counts to
    the 4+8 tile caps.
  - Phase 2, two passes (A then B). Per 128-edge tile: edge MLP via PE
    matmuls, LeakyReLU on the scalar engine, product on DVE (batched once
    per gather run), scatter-add = one-hot matmul into a PSUM window
    [128 slots x 128 feat]. One-hots are precomputed on host and DMA'd in
    on the sync queue (no DVE is_equal). Pass-A windows stash partials in
    SBUF; pass-B adds and stores.
  - Gathers (InstDMAGatherAnt) round-robin on 4 SWDGE queues; the gpsimd
    queue is reserved for gathers so gather issue is never blocked.
    Phase-1 loads + in-loop phase-2 input loads go on the sync queue; the
    first runs' inputs and idx table are prefetched on the scalar queue
    so phase 1 starts immediately. Table-B phase-1 slabs are interleaved
    (front-loaded) with pass-A compute in PE program order.
"""

import os
import sys
import types

import numpy as np

import concourse.bass as bass
import concourse.tile as tile
from concourse.tile import add_dep_helper
from concourse import bacc, mybir
from concourse.bass_utils import run_bass_kernel_spmd

# ---- problem constants (hardcoded per spec) ----
N_NODES = 50000
D_NODE = 128
D_EDGE = 32
D_HID = 128
N_CORES = 8
NPC = N_NODES // N_CORES          # nodes per core = 6250
P = 128                           # partitions
W_PER_CORE = (NPC + P - 1) // P   # 49 windows per core
LAST_W_CAP = NPC - (W_PER_CORE - 1) * P   # 106 node slots in window 48
NEG_SLOPE = 0.01
PAD_SLOT = 300.0                  # slot value that never matches 0..127

N_PAD = 50176                     # 49 x 1024
HALF_A = 17 * 1024                # msg table A rows (17408)
HALF_B = 32 * 1024                # msg table B rows (32768, int16 max fit)
HALVES = (HALF_A, HALF_B)
CAPS = (512, 1024)                # per-window edge caps (4 + 8 tiles)
TPBS = (HALF_A // P, HALF_B // P)  # 136, 256 (partition-major tables)

RUN_MAX = 8          # max tiles per dma_gather (1024 idxs; ring cap)
N_QUEUES = 4         # SWDGE queues for parallel desc-gen
PREFETCH = 6         # phase-2 input runs loaded before phase-1 loads

F32 = mybir.dt.float32
F16 = mybir.dt.float16
I16 = mybir.dt.int16
DT = F16
DT_NP = np.float16

TRACE = False
LAST_RESULT = None

_PROGRAM_CACHE = {}


def _ensure_ntff_hook():
    """Provide antenv.axon_hooks if this image's antenv lacks it, and
    register the ctypes NTFF profiling hook so trace=True works."""
    try:
        from antenv.axon_hooks import get_axon_ntff_profile_hook  # noqa: F401
        return
    except ImportError:
        pass
    mod = types.ModuleType("antenv.axon_hooks")
    _hook = [None]
    mod.set_axon_ntff_profile_hook = lambda h: _hook.__setitem__(0, h)
    mod.get_axon_ntff_profile_hook = lambda: _hook[0]
    sys.modules["antenv.axon_hooks"] = mod
    import antenv

    antenv.axon_hooks = mod
    try:
        from trn_agent_boot.trn_boot import _ntff_profile_via_ctypes

        mod.set_axon_ntff_profile_hook(
            _ntff_profile_via_ctypes("/opt/axon/libaxon_pjrt.so")
        )
    except Exception:
        pass


# --------------------------------------------------------------------------
# host-side balancing permutations
# --------------------------------------------------------------------------

def cdiv(a, b):
    return (a + b - 1) // b


def _balance_dest(seg_i):
    """Permutation pi (orig node id -> new id) equalizing per-(core,window)
    edge loads via LPT greedy. New id = core*NPC + win*P + slot."""
    import heapq

    deg = np.bincount(seg_i, minlength=N_NODES).astype(np.int64)
    order = np.argsort(-deg, kind="stable")
    NB = N_CORES * W_PER_CORE
    caps = np.empty(NB, dtype=np.int64)
    for c in range(N_CORES):
        for w in range(W_PER_CORE):
            caps[c * W_PER_CORE + w] = P if w < W_PER_CORE - 1 else LAST_W_CAP
    fill = np.zeros(NB, dtype=np.int64)
    loads = np.zeros(NB, dtype=np.int64)
    heap = [(0, int(b)) for b in range(NB)]
    heapq.heapify(heap)
    pi = np.empty(N_NODES, dtype=np.int64)
    for n in order:
        while True:
            load, b = heapq.heappop(heap)
            if fill[b] >= caps[b]:
                continue
            if load != loads[b]:
                heapq.heappush(heap, (int(loads[b]), b))
                continue
            break
        c, w = divmod(b, W_PER_CORE)
        pi[n] = c * NPC + w * P + fill[b]
        fill[b] += 1
        loads[b] += deg[n]
        if fill[b] < caps[b]:
            heapq.heappush(heap, (int(loads[b]), b))
    assert (fill == caps).all()
    return pi


def _assign_tables(seg2, idx_j):
    """Per-core A/B table assignment of source nodes (A:B edge ratio 1:2).

    Returns half[e] (0 if edge's source is in table A for its core) and
    row_of[c] : [N_NODES] -> row in 0..N_PAD (A: 0..HALF_A-1, B: HALF_A..),
    node_at_row[c] : [N_PAD] node id at row (-1 = pad).
    """
    core = seg2 // NPC
    win = (seg2 % NPC) // P
    E = len(seg2)
    half = np.zeros(E, dtype=np.int64)
    node_at_row = np.full((N_CORES, N_PAD), -1, dtype=np.int64)
    row_of = np.full((N_CORES, N_NODES), -1, dtype=np.int64)
    CAP_A, CAP_B = CAPS

    for c in range(N_CORES):
        m = np.flatnonzero(core == c)
        srcs = idx_j[m]
        wins = win[m]
        order = np.argsort(srcs, kind="stable")
        s_sorted = srcs[order]
        w_sorted = wins[order]
        uniq, starts = np.unique(s_sorted, return_index=True)
        counts = np.diff(np.append(starts, len(s_sorted)))
        A_cnt = np.zeros(W_PER_CORE, np.int64)
        B_cnt = np.zeros(W_PER_CORE, np.int64)
        sideA = np.zeros(len(uniq), dtype=bool)

        multi = np.flatnonzero(counts > 1)
        multi = multi[np.argsort(-counts[multi], kind="stable")]
        for ui in multi:
            s0 = starts[ui]
            ws = w_sorted[s0:s0 + counts[ui]]
            uw, uc = np.unique(ws, return_counts=True)
            fa = (A_cnt[uw] + uc).max() / CAP_A
            fb = (B_cnt[uw] + uc).max() / CAP_B
            okA = fa <= 1.0
            okB = fb <= 1.0
            useA = fa <= fb if (okA == okB) else okA
            if useA:
                A_cnt[uw] += uc
                sideA[ui] = True
            else:
                B_cnt[uw] += uc

        # single-edge sources: per window, top up A toward tot/3
        single = np.flatnonzero(counts == 1)
        sw = w_sorted[starts[single]]
        for w in range(W_PER_CORE):
            sel = single[sw == w]
            n_s = len(sel)
            tot = A_cnt[w] + B_cnt[w] + n_s
            want_a = int(np.clip(tot // 3 - A_cnt[w], 0, n_s))
            want_a = min(want_a, max(CAP_A - A_cnt[w], 0))
            spill_b = (B_cnt[w] + n_s - want_a) - CAP_B
            if spill_b > 0:
                want_a = min(want_a + spill_b, n_s)
            if want_a:
                sideA[sel[:want_a]] = True
                A_cnt[w] += want_a
            B_cnt[w] += n_s - want_a

        # tables hold ONLY this core's active sources (rows above the
        # active count are never gathered, so their slabs are skipped)
        a_list = uniq[sideA]
        b_list = uniq[~sideA]
        assert len(a_list) <= HALF_A and len(b_list) <= HALF_B, (
            f"core {c}: |A|={len(a_list)} |B|={len(b_list)}"
        )
        node_at_row[c, :len(a_list)] = a_list
        node_at_row[c, HALF_A:HALF_A + len(b_list)] = b_list
        row_of[c, a_list] = np.arange(len(a_list))
        row_of[c, b_list] = HALF_A + np.arange(len(b_list))
        half[m] = (row_of[c, srcs] >= HALF_A).astype(np.int64)

    # slabs actually needed per table (max active count over cores)
    n_used = [
        int((node_at_row[:, :HALF_A] >= 0).sum(axis=1).max()),
        int((node_at_row[:, HALF_A:] >= 0).sum(axis=1).max()),
    ]
    n_slabs = (cdiv(n_used[0], 1024), cdiv(n_used[1], 1024))
    return half, node_at_row, row_of, n_slabs


# --------------------------------------------------------------------------
# host-side schedule + packing
# --------------------------------------------------------------------------

class Schedule:
    """Common (all-core) static schedule baked into the program.

    Tile sequence = pass A (all windows, table 0) then pass B (table 1).
    """

    def __init__(self, seg2, half):
        core = seg2 // NPC
        local = seg2 - core * NPC
        win = local // P

        cnt = np.zeros((2, N_CORES, W_PER_CORE), dtype=np.int64)
        for c in range(N_CORES):
            m = core == c
            for h in (0, 1):
                cnt[h, c] = np.bincount(
                    win[m & (half == h)], minlength=W_PER_CORE
                )
        a_tiles = np.maximum(cdiv(cnt[0].max(axis=0), P), 1)
        b_tiles = np.maximum(cdiv(cnt[1].max(axis=0), P), 1)

        win_of, table_of = [], []
        self.block_start = np.zeros((W_PER_CORE, 2), dtype=np.int64)
        self.block_tiles = np.zeros((W_PER_CORE, 2), dtype=np.int64)
        for tab in (0, 1):
            nts = a_tiles if tab == 0 else b_tiles
            for w in range(W_PER_CORE):
                self.block_start[w, tab] = len(win_of)
                self.block_tiles[w, tab] = nts[w]
                for _ in range(int(nts[w])):
                    win_of.append(w)
                    table_of.append(tab)
        win_of = np.array(win_of, dtype=np.int64)
        table_of = np.array(table_of, dtype=np.int64)
        T = len(win_of)
        first_of = np.zeros(T, dtype=bool)
        last_of = np.zeros(T, dtype=bool)
        for w in range(W_PER_CORE):
            for tab in (0, 1):
                s = int(self.block_start[w, tab])
                n = int(self.block_tiles[w, tab])
                first_of[s] = True
                last_of[s + n - 1] = True

        self.T = T
        self.win_of = win_of
        self.table_of = table_of
        self.first_of = first_of
        self.last_of = last_of
        self.core, self.local, self.win, self.half = core, local, win, half
        self.n_a_tiles = int(a_tiles.sum())

        # gather runs: same-table spans capped at RUN_MAX
        runs = []
        t = 0
        while t < T:
            tab = self.table_of[t]
            e = t
            while e < T and self.table_of[e] == tab and e - t < RUN_MAX:
                e += 1
            runs.append((int(tab), t, e - t))
            t = e
        self.runs = runs

        # chunks: <=4-tile pieces within runs
        chunks = []
        for ri, (tab, t0, L) in enumerate(runs):
            t = t0
            while t < t0 + L:
                nt = min(4, t0 + L - t)
                chunks.append((ri, t, nt))
                t += nt
        self.chunks = chunks

    def key(self):
        return (
            tuple(self.win_of.tolist()),
            tuple(self.table_of.tolist()),
            self.n_slabs,
        )


def _pack_core(c, S, edge, idx_j, row_of_c):
    """Per-core padded arrays following the common schedule."""
    T = S.T
    perm = np.full(T * P, -1, dtype=np.int64)
    for w in range(W_PER_CORE):
        for tab in (0, 1):
            sel = np.flatnonzero(
                (S.core == c) & (S.win == w) & (S.half == tab)
            )
            n = len(sel)
            s0 = S.block_start[w, tab] * P
            cap = S.block_tiles[w, tab] * P
            assert n <= cap, f"schedule overflow c={c} w={w} tab={tab}"
            perm[s0:s0 + n] = sel

    valid = perm >= 0
    pidx = np.where(valid, perm, 0)
    tab_of_pos = S.table_of[np.arange(T * P) // P]

    # idx16 [128, T*8]: per tile-order flattening i=t*128+p -> [i%16, i//16]
    loc = (row_of_c[idx_j[pidx]] - tab_of_pos * HALF_A).astype(np.int64)
    loc[~valid] = 0
    tpb = np.where(tab_of_pos == 0, TPBS[0], TPBS[1])
    assert (loc >= 0).all() and (loc < np.where(tab_of_pos == 0, HALF_A, HALF_B)).all()
    # msg tables partition-major: row j at position (j%128)*TPB + j//128
    loc = (loc % P) * tpb + loc // P
    assert loc.max() <= 32767
    idx16 = loc.astype(np.int16).reshape(T * 8, 16).T  # [16, T*8]
    idx16 = np.tile(idx16, (8, 1)).copy()              # [128, T*8]

    # chunk-packed edge features, slots -> host-built one-hots
    n_ch = len(S.chunks)
    edgeT = np.zeros((n_ch, D_EDGE, 512), dtype=DT_NP)
    slots_all = S.local[pidx] % P
    ev = np.zeros((T * P, D_EDGE), dtype=DT_NP)
    ev[valid] = edge[pidx[valid]]
    for k, (ri, t0, nt) in enumerate(S.chunks):
        edgeT[k, :, :nt * P] = ev[t0 * P:(t0 + nt) * P].T

    # one-hot, edge-position-major: ohT[p, t*128 + s] = (slot(t,p) == s)
    oh = (slots_all[:, None] == np.arange(P)[None, :])
    oh &= valid[:, None]
    ohT = np.ascontiguousarray(
        oh.reshape(T, P, P).transpose(1, 0, 2).reshape(P, T * P)
    ).astype(DT_NP)
    return edgeT, ohT, idx16


# --------------------------------------------------------------------------
# device program
# --------------------------------------------------------------------------

def _build_program(S):
    T = S.T
    n_ch = len(S.chunks)
    dbg_skip_p1 = os.environ.get("K_SKIP_P1") == "1"
    dbg_skip_gather = os.environ.get("K_SKIP_GATHER") == "1"
    dbg_no_gate = os.environ.get("K_NO_GATE") == "1"

    nc = bacc.Bacc(
        "TRN2", target_bir_lowering=False, debug=False, num_devices=N_CORES,
        num_swdge_queues=N_QUEUES,
    )

    # ---- I/O ----
    nodeT_h = nc.dram_tensor("nodeT", [P, N_PAD], DT, kind="ExternalInput").ap()
    Wn_h = nc.dram_tensor("Wn", [D_NODE, D_HID], DT, kind="ExternalInput").ap()
    We1_h = nc.dram_tensor("We1p", [D_EDGE, D_HID], DT, kind="ExternalInput").ap()
    We2_h = nc.dram_tensor("We2", [D_HID, D_HID], DT, kind="ExternalInput").ap()
    be1_h = nc.dram_tensor("be1c", [P, 1], F32, kind="ExternalInput").ap()
    be2_h = nc.dram_tensor("be2bc", [P, 512], F32, kind="ExternalInput").ap()
    edgeT_h = nc.dram_tensor(
        "edgeT", [n_ch, D_EDGE, 512], DT, kind="ExternalInput"
    ).ap()
    ohT_h = nc.dram_tensor(
        "ohT", [P, T * P], DT, kind="ExternalInput"
    ).ap()
    idx16_h = nc.dram_tensor(
        "idx16", [P, T * 8], I16, kind="ExternalInput"
    ).ap()
    out_h = nc.dram_tensor(
        "out", [W_PER_CORE * P, D_HID], F32, kind="ExternalOutput"
    ).ap()

    msg_h = [
        nc.dram_tensor("msgA", [HALF_A, D_HID], DT).ap(),
        nc.dram_tensor("msgB", [HALF_B, D_HID], DT).ap(),
    ]

    LR = mybir.ActivationFunctionType.Prelu

    with tile.TileContext(nc) as tc:
        with tc.tile_pool(name="consts", bufs=1) as cpool:
            Wn_sb = cpool.tile([D_NODE, D_HID], DT)
            nc.sync.dma_start(Wn_sb[:], Wn_h[:])
            We1_sb = cpool.tile([D_EDGE, D_HID], DT)
            We2_sb = cpool.tile([D_HID, D_HID], DT)
            be1_sb = cpool.tile([P, 1], F32)
            be2_sb = cpool.tile([P, 512], F32)
            ix_all = cpool.tile([P, T * 8], I16)
            nc.scalar.dma_start(ix_all[:], idx16_h[:])

            with (
                tc.tile_pool(name="p2_in", bufs=8) as p2in,
                tc.tile_pool(name="p2_oh", bufs=PREFETCH + 4) as ohpool,
            ):
                # prefetch the first PREFETCH runs' phase-2 inputs ahead of
                # the phase-1 load train on the sync queue
                chunks_by_run = {}
                for k, (ri, t0, nt) in enumerate(S.chunks):
                    chunks_by_run.setdefault(ri, []).append((k, t0, nt))

                et_tiles, oh_tiles = {}, {}

                def load_run_inputs(ri, eng):
                    tab, rt0, L = S.runs[ri]
                    rchunks = chunks_by_run[ri]
                    k0 = rchunks[0][0]
                    nk = len(rchunks)
                    et = p2in.tile([D_EDGE, 2 * 512], DT, tag="edgeT")
                    eng.dma_start(
                        et[:, :nk * 512].rearrange("e (k c) -> e k c", k=nk),
                        edgeT_h[k0:k0 + nk, :, :].rearrange(
                            "k e c -> e k c"
                        ),
                    )
                    oh = ohpool.tile([P, RUN_MAX * P], DT, tag="oh")
                    eng.dma_start(
                        oh[:, :L * P], ohT_h[:, rt0 * P:(rt0 + L) * P]
                    )
                    et_tiles[ri] = et
                    oh_tiles[ri] = oh

                # prefetch on the scalar queue: the sync queue must start
                # the phase-1 load train immediately
                for ri in range(min(PREFETCH, len(S.runs))):
                    load_run_inputs(ri, nc.scalar)

                slabs = []
                for tab in (0, 1):
                    for k in range(S.n_slabs[tab]):
                        slabs.append((tab, k * 1024))
                if dbg_skip_p1:
                    slabs = []
                p1_stores = [[], []]
                msg_sems = [
                    nc.alloc_semaphore("msgA_done"),
                    nc.alloc_semaphore("msgB_done"),
                ]

                # ---- phase 1 + 2 (B slabs interleaved with pass A) ----
                with (
                    tc.tile_pool(name="p1_in", bufs=12) as p1in,
                    tc.tile_pool(name="p1_stage", bufs=8) as p1st,
                    tc.tile_pool(name="p2_g", bufs=16) as p2g,
                    tc.tile_pool(name="p2_mid", bufs=6) as p2mid,
                    tc.tile_pool(name="p2_acc", bufs=1) as accp,
                    tc.tile_pool(name="big_psum", bufs=2, space="PSUM") as bigps,
                    tc.tile_pool(name="h2_psum", bufs=2, space="PSUM") as h2ps,
                    tc.tile_pool(name="out_psum", bufs=2, space="PSUM") as outps,
                    tc.tile_pool(name="out_stage", bufs=3) as outst,
                ):
                    def emit_slab(g):
                        tab, r0 = slabs[g]
                        col0 = tab * HALF_A + r0
                        nt_sb = p1in.tile([P, 1024], DT, tag="nodeT")
                        nc.sync.dma_start(
                            nt_sb[:], nodeT_h[:, col0:col0 + 1024]
                        )
                        ps = bigps.tile([P, 1024], F32, tag="big")
                        for t in range(1024 // P):
                            nc.tensor.matmul(
                                ps[:, t * P:(t + 1) * P],
                                lhsT=nt_sb[:, t * P:(t + 1) * P],
                                rhs=Wn_sb[:],
                                start=True,
                                stop=True,
                            )
                        stage = p1st.tile([P, 1024], DT, tag="p1stage")
                        if g % 2 == 0:
                            nc.vector.tensor_copy(stage[:], ps[:])
                        else:
                            nc.scalar.activation(
                                stage[:], ps[:],
                                mybir.ActivationFunctionType.Copy,
                            )
                        # partition-major: row j -> (j%128)*TPB + j//128
                        t0 = r0 // P
                        dst = msg_h[tab][:].rearrange(
                            "(p t) f -> p t f", t=TPBS[tab]
                        )[:, t0:t0 + 8, :]
                        srcap = stage[:].rearrange("p (t f) -> p t f", t=8)
                        st_inst = nc.scalar.dma_start(dst, srcap)
                        p1_stores[tab].append(st_inst.ins)

                    sem_target = [0, 0]

                    def store_barrier(tab):
                        # Cycling the p1stage ring with dummy writes forces
                        # pool-WAR waits on the last 8 stores' completions;
                        # ring recycling orders all earlier stores before
                        # those transitively. The then_inc on the dummy
                        # compute ops is reliable (unlike DMA then_inc or
                        # shared-lane dep counting).
                        nc.scalar.drain(fusable=False).then_inc(
                            msg_sems[tab], 1
                        )
                        n = min(8, len(p1_stores[tab]))
                        for _ in range(n):
                            dmy = p1st.tile([P, 1024], DT, tag="p1stage")
                            nc.vector.tensor_copy(dmy[:1, :1], Wn_sb[:1, :1])
                        # in-order DVE: this inc fires after all dummies
                        nc.vector.sem_inc(msg_sems[tab], 1)
                        sem_target[tab] = 2

                    n_a_slabs = S.n_slabs[0]
                    for g in range(min(n_a_slabs, len(slabs))):
                        emit_slab(g)
                    emitted = [min(n_a_slabs, len(slabs))]
                    if p1_stores[0]:
                        store_barrier(0)
                    # phase-2 consts: issued after the A-slab load train,
                    # well before first use (~65us)
                    nc.sync.dma_start(We1_sb[:], We1_h[:])
                    nc.sync.dma_start(We2_sb[:], We2_h[:])
                    nc.sync.dma_start(be1_sb[:], be1_h[:])
                    nc.sync.dma_start(be2_sb[:], be2_h[:])

                    def emit_b_slabs(upto):
                        while emitted[0] < min(upto, len(slabs)):
                            emit_slab(emitted[0])
                            emitted[0] += 1
                            if emitted[0] == len(slabs):
                                store_barrier(1)
                    cur_out = {}
                    acc = {}
                    reg_full = nc.gpsimd.to_reg(RUN_MAX * P)

                    first_run_of_tab = {}
                    for ri, (tab, rt0, L) in enumerate(S.runs):
                        if tab not in first_run_of_tab:
                            first_run_of_tab[tab] = ri
                    n_a_runs = max(first_run_of_tab.get(1, len(S.runs)), 1)
                    n_b_slabs = len(slabs) - emitted[0]

                    def emit_scatter(job):
                        ri, tab, rt0, L, oh_run, pr_run = job
                        # out_w[s,f] += onehot[:,t].T @ product[:,t]
                        for t in range(L):
                            i = rt0 + t
                            w = int(S.win_of[i])
                            if S.first_of[i]:
                                cur_out[w] = outps.tile(
                                    [P, P], F32, tag="outp",
                                    name=f"outp_w{w}t{tab}"
                                )
                            nc.tensor.matmul(
                                cur_out[w][:],
                                lhsT=oh_run[:, t * P:(t + 1) * P],
                                rhs=pr_run[:, t * P:(t + 1) * P],
                                start=bool(S.first_of[i]),
                                stop=bool(S.last_of[i]),
                            )
                            if S.last_of[i]:
                                if tab == 0:
                                    a = accp.tile(
                                        [P, P], F32, tag=f"acc_w{w}",
                                        name=f"acc_w{w}"
                                    )
                                    nc.vector.tensor_copy(a[:], cur_out[w][:])
                                    acc[w] = a
                                else:
                                    st = outst.tile(
                                        [P, P], F32, tag="outstage",
                                        name=f"outst_w{w}"
                                    )
                                    nc.vector.tensor_tensor(
                                        st[:], in0=cur_out[w][:],
                                        in1=acc[w][:],
                                        op=mybir.AluOpType.add,
                                    )
                                    nc.sync.dma_start(
                                        out_h[w * P:(w + 1) * P, :], st[:]
                                    )
                                del cur_out[w]

                    pending = []
                    for ri, (tab, rt0, L) in enumerate(S.runs):
                        if tab == 0:
                            # interleave table-B phase-1 slabs with pass A:
                            # ~10 before run 0 (PE is head-of-line blocked
                            # on gather-0 data there), the rest spread 2x
                            # front-loaded so msgB is ready when pass-A
                            # gathers drain
                            base = min(20, n_b_slabs)
                            quota = n_a_slabs + base + (
                                2 * (ri + 1) * (n_b_slabs - base)
                            ) // max(n_a_runs, 1)
                            emit_b_slabs(quota)
                        else:
                            emit_b_slabs(len(slabs))

                        if ri == first_run_of_tab.get(tab):
                            # gpsimd-queue barrier on the store-completion
                            # proof (drain + stage-ring WAR dummies)
                            if p1_stores[tab]:
                                nc.gpsimd.wait_ge(
                                    msg_sems[tab], sem_target[tab]
                                )
                        G = p2g.tile(
                            [P, RUN_MAX * P], DT, tag="G", name=f"G_r{ri}"
                        )
                        if dbg_skip_gather:
                            nc.gpsimd.memset(G[:, :L * P], 0.5)
                        else:
                            g_inst = nc.gpsimd.dma_gather(
                                G[:, :L * P].rearrange(
                                    "p (g f) -> p g f", f=P
                                ),
                                msg_h[tab][:],
                                ix_all[:, rt0 * 8:(rt0 + L) * 8],
                                num_idxs=L * P,
                                num_idxs_reg=(
                                    reg_full if L == RUN_MAX else L * P
                                ),
                                elem_size=P,
                                elem_step=P,
                                queue_num=ri % N_QUEUES,
                            )
                            # ordering vs msg stores is enforced by the
                            # wait_ge barrier above; per-store dep edges
                            # lower to shared-lane counts that fire early
                            # (races) or late (stalls) and must not be used


                        if ri not in et_tiles:
                            load_run_inputs(ri, nc.sync)
                        et_run = et_tiles.pop(ri)
                        oh_run = oh_tiles.pop(ri)
                        if ri + PREFETCH < len(S.runs):
                            load_run_inputs(ri + PREFETCH, nc.sync)

                        eh_run = p2mid.tile([P, RUN_MAX * P], DT, tag="eh")
                        pr_run = p2mid.tile([P, RUN_MAX * P], DT, tag="pr")
                        y_run = p2mid.tile([P, RUN_MAX * P], DT, tag="y")

                        # h1 = lrelu(edge @ We1 + be1), [h x e], whole run
                        # (matmul output must stay within one PSUM bank ->
                        #  one 512-col matmul per chunk)
                        ps1 = bigps.tile([P, 1024], F32, tag="big")
                        for (k, t0, nt) in chunks_by_run[ri]:
                            kk = k - chunks_by_run[ri][0][0]
                            nc.tensor.matmul(
                                ps1[:, kk * 512:kk * 512 + nt * P],
                                lhsT=We1_sb[:],
                                rhs=et_run[:, kk * 512:kk * 512 + nt * P],
                                start=True,
                                stop=True,
                            )
                        h1f = p2mid.tile([P, RUN_MAX * P], DT, tag="h1f")
                        nc.scalar.activation(
                            h1f[:, :L * P], ps1[:, :L * P], LR,
                            bias=be1_sb[:], scale=1.0, alpha=NEG_SLOPE,
                        )

                        for (k, t0, nt) in chunks_by_run[ri]:
                            ncols = nt * P
                            kk = k - chunks_by_run[ri][0][0]

                            # h2 = h1.T @ We2 + be2, edge-major [e x h]
                            ps2 = h2ps.tile([P, 512], F32, tag="h2ps")
                            for t in range(nt):
                                tt = kk * 4 + t
                                nc.tensor.matmul(
                                    ps2[:, t * P:(t + 1) * P],
                                    lhsT=h1f[:, tt * P:(tt + 1) * P],
                                    rhs=We2_sb[:],
                                    start=True,
                                    stop=True,
                                )
                            # bias add downcasts to fp16 SBUF (cheap act in)
                            nc.vector.tensor_tensor(
                                y_run[:, kk * 512:kk * 512 + ncols],
                                in0=ps2[:, :ncols],
                                in1=be2_sb[:, :ncols], op=mybir.AluOpType.add,
                            )

                        # eh = lrelu(y), whole run in one scalar op
                        nc.scalar.activation(
                            eh_run[:, :L * P], y_run[:, :L * P], LR,
                            scale=1.0, alpha=NEG_SLOPE,
                        )

                        # product = gathered msg * edge_h, whole run at once
                        nc.vector.tensor_tensor(
                            pr_run[:, :L * P],
                            in0=G[:, :L * P],
                            in1=eh_run[:, :L * P],
                            op=mybir.AluOpType.mult,
                        )

                        # scatter is emitted one run late (software
                        # pipeline): the in-order PE queue would otherwise
                        # head-of-line block the next run's edge MLP on
                        # this run's gather data
                        pending.append((ri, tab, rt0, L, oh_run, pr_run))
                        if len(pending) > 3:
                            emit_scatter(pending.pop(0))

                    while pending:
                        emit_scatter(pending.pop(0))

    nc.compile()
    return nc


# --------------------------------------------------------------------------
# entry point
# --------------------------------------------------------------------------

def kernel(node, edge, Wn, We1, be1, We2, be2, seg_i, idx_j):
    global LAST_RESULT
    node = np.asarray(node, dtype=np.float32)
    edge = np.asarray(edge, dtype=np.float32)
    Wn = np.asarray(Wn, dtype=np.float32)
    We1 = np.asarray(We1, dtype=np.float32)
    be1 = np.asarray(be1, dtype=np.float32)
    We2 = np.asarray(We2, dtype=np.float32)
    be2 = np.asarray(be2, dtype=np.float32)
    seg_i = np.asarray(seg_i, dtype=np.int32).astype(np.int64)
    idx_j = np.asarray(idx_j, dtype=np.int32).astype(np.int64)

    pi = _balance_dest(seg_i)
    seg2 = pi[seg_i]
    half, node_at_row, row_of, n_slabs = _assign_tables(seg2, idx_j)
    S = Schedule(seg2, half)
    S.n_slabs = n_slabs
    key = S.key()
    if key not in _PROGRAM_CACHE:
        _PROGRAM_CACHE[key] = _build_program(S)
    nc = _PROGRAM_CACHE[key]

    common = {
        "Wn": Wn.astype(DT_NP),
        "We1p": We1.astype(DT_NP),
        "We2": We2.astype(DT_NP),
        "be1c": be1.reshape(P, 1).copy(),
        "be2bc": np.broadcast_to(
            np.tile(be2, 4), (P, 512)
        ).astype(np.float32).copy(),
    }
    nodeT_f16 = node.T.astype(DT_NP)   # [128, N_NODES]
    in_maps = []
    for c in range(N_CORES):
        edgeT, ohT, idx16 = _pack_core(c, S, edge, idx_j, row_of[c])
        nodeT = np.zeros((P, N_PAD), dtype=DT_NP)
        rows = node_at_row[c]
        m = rows >= 0
        nodeT[:, m] = nodeT_f16[:, rows[m]]
        mp = dict(common)
        mp["nodeT"] = nodeT
        mp["edgeT"] = edgeT
        mp["ohT"] = ohT
        mp["idx16"] = idx16
        in_maps.append(mp)

    if TRACE:
        _ensure_ntff_hook()
    res = run_bass_kernel_spmd(
        nc, in_maps, list(range(N_CORES)), trace=TRACE
    )
    LAST_RESULT = res
    out_cat = np.concatenate(
        [res.results[c]["out"][:NPC] for c in range(N_CORES)], axis=0
    )
    return out_cat[pi].astype(np.float32)
